# revision 5
# baseline (speedup 1.0000x reference)
"""Trainium2 Bass kernel for factorized spatial attention (nn_Attention_50379966382361).

Reference computation (per batch b, frame f):
    qkv = x @ Wqkv.T ; split into q,k,v heads (8 heads, hd=64)
    attn = softmax(q @ k.T * hd**-0.5) over spatial tokens (n=784) within the frame
    out  = attn @ v ; merge heads ; y = out @ Wproj.T + bproj

Sharding: data-parallel over the 32 (b, f) frames -> 4 frames per core, weights
replicated.  All tensors are staged channel-major on device (tokens on the free
axis) so no on-device transposes are needed:
  - scoresT[j, i] is computed directly via lhsT=kT, rhs=qT (2 heads row-packed
    in the 128x128 PE array since hd=64).
  - exp via ScalarE with the 1/sqrt(hd) scale folded into the activation scale
    (no max subtraction: |scale*s| < ~1.5 for these input statistics).
  - attn@v uses lhsT=v(token-major, produced directly by a second QKV matmul
    orientation), rhs=expT, 2 heads column-packed.
  - softmax denominators via a ones-column matmul into spare PSUM partitions;
    1/sums broadcast across partitions with a tiny K=33 indicator matmul.
"""

import numpy as np

import concourse.bass as bass
import concourse.mybir as mybir
import concourse.tile as tile
from concourse.bass_utils import run_bass_kernel_spmd

B, F, N, VD, D, H = 2, 16, 784, 512, 512, 8
HD = D // H                      # 64
NCORES = 8
FPC = (B * F) // NCORES          # frames per core = 4
JT = 112                         # token tile (7 * 112 = 784, no tail)
NJ = N // JT                     # 7
CHUNKS = ((0, 512), (512, 272))  # free-dim chunks of 784 (PSUM bank = 512 fp32)
FP32 = mybir.dt.float32
AF = mybir.ActivationFunctionType


def _split_ctrl_waits(nc):
    """This walrus build only accepts a single sync-wait per instruction
    (setupSyncWait raises "Too many sync wait commands"), while Tile's
    scheduler aggregates several.  Move the excess waits onto NoOps inserted
    just before (same engine; engines execute in order, so waiting earlier
    on the same queue is equivalent)."""
    for f in nc.m.functions:
        for blk in f.blocks:
            new_list, changed = [], False
            for inst in blk.instructions:
                si = inst.sync_info
                if si is not None and len(si.on_wait) > 1:
                    waits = list(si.on_wait)
                    for w_i, w in enumerate(waits[:-1]):
                        new_list.append(
                            mybir.InstNoOp(
                                name=f"{inst.name}-waitsplit{w_i}",
                                ins=[],
                                outs=[],
                                engine=inst.engine,
                                bass_nofuse=True,
                                sync_info=mybir.SyncInfo(on_wait=[w], on_update=[]),
                            )
                        )
                    inst.sync_info = mybir.SyncInfo(
                        on_wait=[waits[-1]], on_update=list(si.on_update)
                    )
                    changed = True
                new_list.append(inst)
            if changed:
                blk.instructions = new_list


def build_nc():
    nc = bass.Bass("TRN2", target_bir_lowering=False, debug=False, num_devices=NCORES)

    xT = nc.declare_dram_parameter("xT", [FPC, VD, N], FP32, isOutput=False)
    WqkvT = nc.declare_dram_parameter("WqkvT", [VD, 3 * D], FP32, isOutput=False)
    WprojT = nc.declare_dram_parameter("WprojT", [D, VD], FP32, isOutput=False)
    bproj = nc.declare_dram_parameter("bproj", [VD], FP32, isOutput=False)
    yT = nc.declare_dram_parameter("yT", [FPC, VD, N], FP32, isOutput=True)

    with tile.TileContext(nc) as tc:
        with (
            tc.tile_pool(name="w", bufs=1) as w_pool,
            tc.tile_pool(name="x", bufs=2) as x_pool,
            tc.tile_pool(name="qk", bufs=5) as qk_pool,
            tc.tile_pool(name="v", bufs=9) as v_pool,
            tc.tile_pool(name="e", bufs=16) as e_pool,
            tc.tile_pool(name="osc", bufs=5) as osc_pool,
            tc.tile_pool(name="r", bufs=3) as r_pool,
            tc.tile_pool(name="y", bufs=5) as y_pool,
            tc.tile_pool(name="mm", bufs=2, space="PSUM") as mm_ps,
            tc.tile_pool(name="oacc", bufs=1, space="PSUM") as out_ps,
            tc.tile_pool(name="sums", bufs=1, space="PSUM") as sum_ps,
        ):
            # ---- constants / weights (once per core) ----
            W1 = w_pool.tile([128, 4, 3 * D], FP32)   # WqkvT, d-major tiles
            nc.sync.dma_start(out=W1, in_=WqkvT.rearrange("(a p) m -> p a m", p=128))
            W2 = w_pool.tile([128, 4, VD], FP32)      # WprojT
            nc.sync.dma_start(out=W2, in_=WprojT.rearrange("(a p) m -> p a m", p=128))
            bias_sb = w_pool.tile([128, 4], FP32)
            nc.sync.dma_start(out=bias_sb, in_=bproj.rearrange("(a p) -> p a", p=128))
            ones_sb = w_pool.tile([JT, 1], FP32)
            nc.vector.memset(ones_sb, 1.0)
            # indicator for the K=33 recip broadcast: row0 -> cols 0:64 (head A),
            # row32 -> cols 64:128 (head B)
            eind = w_pool.tile([33, 128], FP32)
            nc.vector.memset(eind, 0.0)
            nc.vector.memset(eind[0:1, 0:HD], 1.0)
            nc.vector.memset(eind[32:33, HD:128], 1.0)

            for fr in range(FPC):
                # ---- load xT for this frame, channel-major ----
                X = x_pool.tile([128, 4, N], FP32, tag="X")
                nc.sync.dma_start(
                    out=X, in_=xT[fr].rearrange("(a p) t -> p a t", p=128)
                )

                # ---- V in token-major layout: v_tok[tt][t, vchan] ----
                v_tok = []
                for tt in range(NJ):
                    psv = mm_ps.tile([JT, VD], FP32, tag="mm")
                    for kt in range(4):
                        nc.tensor.matmul(
                            psv,
                            X[:, kt, tt * JT : (tt + 1) * JT],
                            W1[:, kt, 2 * D : 3 * D],
                            start=(kt == 0),
                            stop=(kt == 3),
                        )
                    vt = v_pool.tile([JT, VD], FP32, tag="vtok")
                    nc.vector.tensor_copy(vt, psv)
                    v_tok.append(vt)

                out_scaled = []
                for p in range(4):  # head pairs (2p, 2p+1)
                    # ---- q/k channel-major tiles for this pair ----
                    qk = {}
                    for name, ot in (("q", p), ("k", 4 + p)):
                        ps = mm_ps.tile([128, N], FP32, tag="mm")
                        for kt in range(4):
                            for c0, cw in CHUNKS:
                                nc.tensor.matmul(
                                    ps[:, c0 : c0 + cw],
                                    W1[:, kt, ot * 128 : (ot + 1) * 128],
                                    X[:, kt, c0 : c0 + cw],
                                    start=(kt == 0),
                                    stop=(kt == 3),
                                )
                        t = qk_pool.tile([128, N], FP32, tag="qkT")
                        nc.vector.tensor_copy(t, ps)
                        qk[name] = t

                    # ---- attention for the pair ----
                    out_acc = out_ps.tile([128, N], FP32, tag="oacc")
                    sums = sum_ps.tile([65, N], FP32, tag="sums")
                    # rows other than 0/32 feed the K=33 recip window; keep
                    # them finite (1.0) so reciprocal stays NaN-free.
                    nc.vector.memset(sums, 1.0)
                    for jt in range(NJ):
                        j0 = jt * JT
                        e_ab = []
                        for h in range(2):  # head within pair
                            hp = h * HD
                            ps_s = mm_ps.tile([JT, N], FP32, tag="mm")
                            for c0, cw in CHUNKS:
                                nc.tensor.matmul(
                                    ps_s[:, c0 : c0 + cw],
                                    qk["k"][hp : hp + HD, j0 : j0 + JT],
                                    qk["q"][hp : hp + HD, c0 : c0 + cw],
                                    start=True,
                                    stop=True,
                                    tile_position=(hp, 0),
                                )
                            e = e_pool.tile([JT, N], FP32, tag="expT")
                            nc.scalar.activation(
                                out=e, in_=ps_s, func=AF.Exp, scale=HD**-0.5
                            )
                            e_ab.append(e)
                        for h in range(2):
                            hp = h * HD
                            for c0, cw in CHUNKS:
                                # attn @ v (col-packed: head A -> psum rows
                                # 0:64, head B -> 64:128)
                                nc.tensor.matmul(
                                    out_acc[hp : hp + HD, c0 : c0 + cw],
                                    v_tok[jt][:, (2 * p + h) * HD : (2 * p + h + 1) * HD],
                                    e_ab[h][:, c0 : c0 + cw],
                                    start=(jt == 0),
                                    stop=(jt == NJ - 1),
                                    tile_position=(0, hp),
                                )
                                # softmax denominators (M=1 rows 0 / 32)
                                nc.tensor.matmul(
                                    sums[32 * h : 32 * h + 1, c0 : c0 + cw],
                                    ones_sb,
                                    e_ab[h][:, c0 : c0 + cw],
                                    start=(jt == 0),
                                    stop=(jt == NJ - 1),
                                    tile_position=(0, 32 * h),
                                )

                    # ---- normalize: out_acc * (1/sums) broadcast over chans ----
                    r_sb = r_pool.tile([65, N], FP32, tag="rsb")
                    nc.vector.reciprocal(out=r_sb, in_=sums)
                    ps_r = mm_ps.tile([128, N], FP32, tag="mm")
                    for c0, cw in CHUNKS:
                        nc.tensor.matmul(
                            ps_r[:, c0 : c0 + cw],
                            eind,
                            r_sb[0:33, c0 : c0 + cw],
                            start=True,
                            stop=True,
                        )
                    r_full = r_pool.tile([128, N], FP32, tag="rfull")
                    nc.vector.tensor_copy(r_full, ps_r)
                    osc = osc_pool.tile([128, N], FP32, tag="osc")
                    nc.vector.tensor_mul(osc, out_acc, r_full)
                    out_scaled.append(osc)

                # ---- output projection + bias ----
                for ot in range(4):
                    ps_y = mm_ps.tile([128, N], FP32, tag="mm")
                    for kt in range(4):
                        for c0, cw in CHUNKS:
                            nc.tensor.matmul(
                                ps_y[:, c0 : c0 + cw],
                                W2[:, kt, ot * 128 : (ot + 1) * 128],
                                out_scaled[kt][:, c0 : c0 + cw],
                                start=(kt == 0),
                                stop=(kt == 3),
                            )
                    yt = y_pool.tile([128, N], FP32, tag="yT")
                    nc.vector.tensor_scalar_add(
                        out=yt, in0=ps_y, scalar1=bias_sb[:, ot : ot + 1]
                    )
                    nc.sync.dma_start(
                        out=yT[fr, ot * 128 : (ot + 1) * 128, :], in_=yt
                    )

    _split_ctrl_waits(nc)
    return nc


_CACHE = {}


def _get_runner():
    """Build the Bass module once and wrap it in a cached sharded jax.jit
    callable (replicates bass2jax.run_bass_via_pjrt but reusable across
    calls, so repeated invocations don't re-lower/re-compile)."""
    if "runner" in _CACHE:
        return _CACHE["runner"]

    import jax
    from jax.experimental.shard_map import shard_map
    from jax.sharding import Mesh, PartitionSpec
    from concourse import bass2jax, mybir as _mybir

    nc = build_nc()
    bass2jax.install_neuronx_cc_hook()
    assert nc.dbg_addr is None
    partition_name = nc.partition_id_tensor.name if nc.partition_id_tensor else None

    in_names, out_names, out_avals, out_shapes = [], [], [], []
    for alloc in nc.m.functions[0].allocations:
        if not isinstance(alloc, _mybir.MemoryLocationSet):
            continue
        name = alloc.memorylocations[0].name
        if alloc.kind == "ExternalInput":
            if name != partition_name:
                in_names.append(name)
        elif alloc.kind == "ExternalOutput":
            shape = tuple(alloc.tensor_shape)
            dtype = _mybir.dt.np(alloc.dtype)
            out_names.append(name)
            out_avals.append(jax.core.ShapedArray(shape, dtype))
            out_shapes.append((shape, dtype))
    n_params = len(in_names)
    all_names = in_names + out_names
    if partition_name is not None:
        all_names = all_names + [partition_name]

    def _body(*args):
        operands = list(args)
        if partition_name is not None:
            operands.append(bass2jax.partition_id_tensor())
        outs = bass2jax._bass_exec_p.bind(
            *operands,
            out_avals=tuple(out_avals),
            in_names=tuple(all_names),
            out_names=tuple(out_names),
            lowering_input_output_aliases=(),
            sim_require_finite=True,
            sim_require_nnan=True,
            nc=nc,
        )
        return tuple(outs)

    devices = jax.devices()[:NCORES]
    mesh = Mesh(np.asarray(devices), ("core",))
    nin = n_params + len(out_names)
    sharded = jax.jit(
        shard_map(
            _body,
            mesh=mesh,
            in_specs=(PartitionSpec("core"),) * nin,
            out_specs=(PartitionSpec("core"),) * len(out_names),
            check_rep=False,
        ),
        donate_argnums=tuple(range(n_params, nin)),
        keep_unused=True,
    )

    def run(in_maps):
        concat_in = [
            np.concatenate([np.asarray(m[name]) for m in in_maps], axis=0)
            for name in in_names
        ]
        concat_zeros = [
            np.zeros((NCORES * s[0], *s[1:]), dt) for s, dt in out_shapes
        ]
        out_arrs = sharded(*concat_in, *concat_zeros)
        return [
            {
                name: np.asarray(out_arrs[i]).reshape(
                    NCORES, *out_shapes[i][0]
                )[c]
                for i, name in enumerate(out_names)
            }
            for c in range(NCORES)
        ]

    _CACHE["runner"] = run
    return run


def kernel(x, Wqkv, Wproj, bproj, spatial=None, f=None, n=None, **_ignored):
    x = np.ascontiguousarray(np.asarray(x, dtype=np.float32))
    Wqkv = np.asarray(Wqkv, dtype=np.float32)
    Wproj = np.asarray(Wproj, dtype=np.float32)
    bp = np.ascontiguousarray(np.asarray(bproj, dtype=np.float32))

    # (b, f*n, d) -> (b*f, d_in, n) channel-major per frame
    xt = np.ascontiguousarray(
        x.reshape(B, F, N, VD).reshape(B * F, N, VD).transpose(0, 2, 1)
    )
    WqkvT = np.ascontiguousarray(Wqkv.T)
    WprojT = np.ascontiguousarray(Wproj.T)

    in_maps = [
        {
            "xT": np.ascontiguousarray(xt[c * FPC : (c + 1) * FPC]),
            "WqkvT": WqkvT,
            "WprojT": WprojT,
            "bproj": bp,
        }
        for c in range(NCORES)
    ]
    results = _get_runner()(in_maps)

    y = np.empty((B * F, N, VD), dtype=np.float32)
    for c in range(NCORES):
        y[c * FPC : (c + 1) * FPC] = results[c]["yT"].transpose(0, 2, 1)
    return y.reshape(B, F * N, VD)


# revision 18
# speedup vs baseline: 51.8276x; 51.8276x over previous
"""Trainium2 Bass kernel for factorized spatial attention (nn_Attention_50379966382361).

Reference computation (per batch b, frame f):
    qkv = x @ Wqkv.T ; split into q,k,v heads (8 heads, hd=64)
    attn = softmax(q @ k.T * hd**-0.5) over spatial tokens (n=784) within the frame
    out  = attn @ v ; merge heads ; y = out @ Wproj.T + bproj

Sharding: data-parallel over the 32 (b, f) frames -> 4 frames per core, weights
replicated.  All tensors are staged channel-major on device (tokens on the free
axis) so no on-device transposes are needed:
  - scoresT[j, i] is computed directly via lhsT=kT, rhs=qT (2 heads row-packed
    in the 128x128 PE array since hd=64).
  - exp via ScalarE with the 1/sqrt(hd) scale folded into the activation scale
    (no max subtraction: |scale*s| < ~1.5 for these input statistics).
  - attn@v uses lhsT=v(token-major, produced directly by a second QKV matmul
    orientation), rhs=expT, 2 heads column-packed.
  - softmax denominators via a ones-column matmul into spare PSUM partitions;
    1/sums broadcast across partitions with a tiny K=33 indicator matmul.
"""

import numpy as np

import concourse.bass as bass
import concourse.mybir as mybir
import concourse.tile as tile
from concourse.bass_utils import run_bass_kernel_spmd

B, F, N, VD, D, H = 2, 16, 784, 512, 512, 8
HD = D // H                      # 64
NCORES = 8
FPC = (B * F) // NCORES          # frames per core = 4
JT = 112                         # token tile (7 * 112 = 784, no tail)
NJ = N // JT                     # 7
CHUNKS = ((0, 512), (512, 272))  # free-dim chunks of 784 (PSUM bank = 512 fp32)
FP32 = mybir.dt.float32
F32R = mybir.dt.float32r
AF = mybir.ActivationFunctionType


def _split_ctrl_waits(nc):
    """This walrus build only accepts a single sync-wait per instruction
    (setupSyncWait raises "Too many sync wait commands"), while Tile's
    scheduler aggregates several.  Move the excess waits onto NoOps inserted
    just before (same engine; engines execute in order, so waiting earlier
    on the same queue is equivalent)."""
    for f in nc.m.functions:
        for blk in f.blocks:
            new_list, changed = [], False
            for inst in blk.instructions:
                si = inst.sync_info
                if si is not None and len(si.on_wait) > 1:
                    waits = list(si.on_wait)
                    for w_i, w in enumerate(waits[:-1]):
                        new_list.append(
                            mybir.InstNoOp(
                                name=f"{inst.name}-waitsplit{w_i}",
                                ins=[],
                                outs=[],
                                engine=inst.engine,
                                bass_nofuse=True,
                                sync_info=mybir.SyncInfo(on_wait=[w], on_update=[]),
                            )
                        )
                    inst.sync_info = mybir.SyncInfo(
                        on_wait=[waits[-1]], on_update=list(si.on_update)
                    )
                    changed = True
                new_list.append(inst)
            if changed:
                blk.instructions = new_list


def build_nc():
    nc = bass.Bass("TRN2", target_bir_lowering=False, debug=False, num_devices=NCORES)

    # host pre-arranges inputs into the on-chip tile layout (partition-major)
    # so every load is a dense contiguous DMA (SWDGE descriptor gen is the
    # startup bottleneck otherwise)
    xT = nc.declare_dram_parameter("xT", [FPC, 128, 4, N], FP32, isOutput=False)
    WqkvT = nc.declare_dram_parameter("WqkvT", [128, 4, 3 * D], FP32, isOutput=False)
    WprojT = nc.declare_dram_parameter("WprojT", [128, 4, VD], FP32, isOutput=False)
    bproj = nc.declare_dram_parameter("bproj", [VD], FP32, isOutput=False)
    yT = nc.declare_dram_parameter("yT", [FPC, VD, N], FP32, isOutput=True)

    # attn@v output columns: head A of a pair at [0, 784), head B at
    # [1024, 1808) of a [65, 2048] psum tile (PSUM-bank aligned chunks).
    BOFF = 1024

    with tile.TileContext(nc) as tc:
        with (
            nc.allow_low_precision(
                reason="float32r matmul operands (TF32-like, ~1.7e-4 rel err)"
            ),
            tc.tile_pool(name="w", bufs=1) as w_pool,
            tc.tile_pool(name="x", bufs=2) as x_pool,
            tc.tile_pool(name="qk", bufs=4) as qk_pool,
            tc.tile_pool(name="v", bufs=8) as v_pool,
            tc.tile_pool(name="e", bufs=10) as e_pool,
            tc.tile_pool(name="osc", bufs=2) as osc_pool,
            tc.tile_pool(name="pb", bufs=5) as pb_pool,
            tc.tile_pool(name="r", bufs=2) as r_pool,
            tc.tile_pool(name="y", bufs=4) as y_pool,
            tc.tile_pool(name="mm", bufs=2, space="PSUM") as mm_ps,
            tc.tile_pool(name="oacc", bufs=1, space="PSUM") as out_ps,
        ):
            # ---- constants / weights (once per core) ----
            # (emitted as four per-k-tile DMAs so frame-0 matmuls can start
            # as soon as their k-slice has landed)
            W1 = w_pool.tile([128, 4, 3 * D], F32R)   # WqkvT, d-major tiles
            for kt in range(4):
                nc.gpsimd.dma_start(out=W1[:, kt, :], in_=WqkvT[:, kt, :])
            W2 = w_pool.tile([128, 4, VD], F32R)      # WprojT, d-major tiles
            nc.gpsimd.dma_start(out=W2, in_=WprojT[:])
            bias_sb = w_pool.tile([128, 4], FP32)
            nc.sync.dma_start(out=bias_sb, in_=bproj.rearrange("(a p) -> p a", p=128))
            # fp32 staging for constants (memset cannot produce float32r)
            ones_f = w_pool.tile([JT, 8, 1], FP32)
            nc.vector.memset(ones_f, 1.0)
            # K=1 lhsT for the 1/sums partition-broadcast: row 64 of [65, 64]
            o65_f = w_pool.tile([65, HD], FP32)
            nc.vector.memset(o65_f, 0.0)
            nc.vector.memset(o65_f[64:65, :], 1.0)
            ones65 = w_pool.tile([65, HD], F32R)
            nc.vector.tensor_copy(ones65, o65_f)

            for fr in range(FPC):
                # ---- load xT for this frame, channel-major ----
                X = x_pool.tile([128, 4, N], F32R, tag="X")
                nc.gpsimd.dma_start(out=X, in_=xT[fr])

                # ---- V token-major with a ones column per head:
                #      vt[:, 65h : 65h+64] = v_h, vt[:, 65h+64] = 1.0 ----
                v_tok = []
                for tt in range(NJ):
                    psv = mm_ps.tile([JT, VD], FP32, tag="mm")
                    for kt in range(4):
                        nc.tensor.matmul(
                            psv,
                            X[:, kt, tt * JT : (tt + 1) * JT],
                            W1[:, kt, 2 * D : 3 * D],
                            start=(kt == 0),
                            stop=(kt == 3),
                        )
                    vt = v_pool.tile([JT, H, 65], F32R, tag="vtok")
                    nc.vector.tensor_copy(
                        vt[:, :, 0:HD], psv.rearrange("p (h c) -> p h c", c=HD)
                    )
                    nc.vector.tensor_copy(vt[:, :, HD : HD + 1], ones_f)
                    v_tok.append(vt)

                def emit_attnv(p, jt, e_ab, out_acc):
                    for h in range(2):
                        for c0, cw in CHUNKS:
                            nc.tensor.matmul(
                                out_acc[0:65, BOFF * h + c0 : BOFF * h + c0 + cw],
                                v_tok[jt][:, 2 * p + h, :],
                                e_ab[h][:, c0 : c0 + cw],
                                start=(jt == 0),
                                stop=(jt == NJ - 1),
                            )

                def emit_normalize(p, out_acc):
                    # rows 0:64 * (1/row 64), both heads of the pair
                    r_sb = r_pool.tile([65, 2, N], F32R, tag="rsb")
                    nc.vector.reciprocal(
                        out=r_sb[64:65, :, :],
                        in_=out_acc[64:65, :].rearrange("p (s q) -> p s q", s=2)[
                            :, :, 0:N
                        ],
                    )
                    r2 = r_pool.tile([HD, 2, N], FP32, tag="r2")
                    for h in range(2):
                        ps_r = mm_ps.tile([HD, N], FP32, tag="mm")
                        for c0, cw in CHUNKS:
                            nc.tensor.matmul(
                                ps_r[:, c0 : c0 + cw],
                                ones65[64:65, :],
                                r_sb[64:65, h, c0 : c0 + cw],
                                start=True,
                                stop=True,
                            )
                        nc.vector.tensor_copy(r2[:, h, :], ps_r)
                    big = pb_pool.tile([128, N], F32R, tag="pb")
                    nc.vector.tensor_mul(
                        big[0:HD, :], out_acc[0:HD, 0:N], r2[:, 0, :]
                    )
                    odd = osc_pool.tile([HD, N], F32R, tag="osc")
                    nc.vector.tensor_mul(
                        odd, out_acc[0:HD, BOFF : BOFF + N], r2[:, 1, :]
                    )
                    nc.sync.dma_start(out=big[HD:128, :], in_=odd)
                    out_osc.append(big)

                out_osc = []
                prev_pair = None  # (p, out_acc) awaiting normalize
                for p in range(4):  # head pairs (2p, 2p+1)
                    # ---- q/k channel-major tiles for this pair ----
                    qk = {}
                    for name, ot in (("q", p), ("k", 4 + p)):
                        ps = mm_ps.tile([128, N], FP32, tag="mm")
                        for kt in range(4):
                            for c0, cw in CHUNKS:
                                nc.tensor.matmul(
                                    ps[:, c0 : c0 + cw],
                                    W1[:, kt, ot * 128 : (ot + 1) * 128],
                                    X[:, kt, c0 : c0 + cw],
                                    start=(kt == 0),
                                    stop=(kt == 3),
                                )
                        t = qk_pool.tile([128, N], F32R, tag="qkT")
                        nc.vector.tensor_copy(t, ps)
                        qk[name] = t

                    if prev_pair is not None:
                        emit_normalize(*prev_pair)
                        prev_pair = None

                    # ---- attention: out rows 0:64 = attn@v, row 64 = sums ----
                    # software-pipelined: attn@v for j-tile jt-1 is emitted
                    # after scores/exp of jt so the in-order PE queue never
                    # heads-of-line blocks on the exp latency.
                    out_acc = out_ps.tile([65, 2048], FP32, tag="oacc")
                    pend = None
                    for jt in range(NJ):
                        j0 = jt * JT
                        e_ab = []
                        for h in range(2):  # head within pair
                            hp = h * HD
                            ps_s = mm_ps.tile([JT, N], FP32, tag="mm")
                            for c0, cw in CHUNKS:
                                nc.tensor.matmul(
                                    ps_s[:, c0 : c0 + cw],
                                    qk["k"][hp : hp + HD, j0 : j0 + JT],
                                    qk["q"][hp : hp + HD, c0 : c0 + cw],
                                    start=True,
                                    stop=True,
                                    tile_position=(hp, 0),
                                )
                            e = e_pool.tile([JT, N], F32R, tag="expT")
                            nc.scalar.activation(
                                out=e, in_=ps_s, func=AF.Exp, scale=HD**-0.5
                            )
                            e_ab.append(e)
                        if pend is not None:
                            emit_attnv(p, pend[0], pend[1], out_acc)
                        pend = (jt, e_ab)
                    emit_attnv(p, pend[0], pend[1], out_acc)
                    prev_pair = (p, out_acc)
                emit_normalize(*prev_pair)

                # ---- output projection + bias (K=64 per head) ----
                for ot in range(4):
                    ps_y = mm_ps.tile([128, N], FP32, tag="mm")
                    for kt in range(4):
                        for c0, cw in CHUNKS:
                            nc.tensor.matmul(
                                ps_y[:, c0 : c0 + cw],
                                W2[:, kt, ot * 128 : (ot + 1) * 128],
                                out_osc[kt][:, c0 : c0 + cw],
                                start=(kt == 0),
                                stop=(kt == 3),
                            )
                    yt = y_pool.tile([128, N], FP32, tag="yT")
                    # bias-add on ScalarE: the ACT engine idles at frame
                    # boundaries while DVE is the local bottleneck
                    nc.scalar.activation(
                        out=yt,
                        in_=ps_y,
                        func=AF.Identity,
                        bias=bias_sb[:, ot : ot + 1],
                    )
                    nc.sync.dma_start(
                        out=yT[fr, ot * 128 : (ot + 1) * 128, :], in_=yt
                    )

    _split_ctrl_waits(nc)
    return nc


_CACHE = {}


def _get_runner():
    """Build the Bass module once and wrap it in a cached sharded jax.jit
    callable (replicates bass2jax.run_bass_via_pjrt but reusable across
    calls, so repeated invocations don't re-lower/re-compile)."""
    if "runner" in _CACHE:
        return _CACHE["runner"]

    import jax
    from jax.experimental.shard_map import shard_map
    from jax.sharding import Mesh, PartitionSpec
    from concourse import bass2jax, mybir as _mybir

    nc = build_nc()
    bass2jax.install_neuronx_cc_hook()
    assert nc.dbg_addr is None
    partition_name = nc.partition_id_tensor.name if nc.partition_id_tensor else None

    in_names, out_names, out_avals, out_shapes = [], [], [], []
    for alloc in nc.m.functions[0].allocations:
        if not isinstance(alloc, _mybir.MemoryLocationSet):
            continue
        name = alloc.memorylocations[0].name
        if alloc.kind == "ExternalInput":
            if name != partition_name:
                in_names.append(name)
        elif alloc.kind == "ExternalOutput":
            shape = tuple(alloc.tensor_shape)
            dtype = _mybir.dt.np(alloc.dtype)
            out_names.append(name)
            out_avals.append(jax.core.ShapedArray(shape, dtype))
            out_shapes.append((shape, dtype))
    n_params = len(in_names)
    all_names = in_names + out_names
    if partition_name is not None:
        all_names = all_names + [partition_name]

    def _body(*args):
        operands = list(args)
        if partition_name is not None:
            operands.append(bass2jax.partition_id_tensor())
        outs = bass2jax._bass_exec_p.bind(
            *operands,
            out_avals=tuple(out_avals),
            in_names=tuple(all_names),
            out_names=tuple(out_names),
            lowering_input_output_aliases=(),
            sim_require_finite=True,
            sim_require_nnan=True,
            nc=nc,
        )
        return tuple(outs)

    devices = jax.devices()[:NCORES]
    mesh = Mesh(np.asarray(devices), ("core",))
    nin = n_params + len(out_names)
    sharded = jax.jit(
        shard_map(
            _body,
            mesh=mesh,
            in_specs=(PartitionSpec("core"),) * nin,
            out_specs=(PartitionSpec("core"),) * len(out_names),
            check_rep=False,
        ),
        donate_argnums=tuple(range(n_params, nin)),
        keep_unused=True,
    )

    def run(in_maps):
        concat_in = [
            np.concatenate([np.asarray(m[name]) for m in in_maps], axis=0)
            for name in in_names
        ]
        concat_zeros = [
            np.zeros((NCORES * s[0], *s[1:]), dt) for s, dt in out_shapes
        ]
        out_arrs = sharded(*concat_in, *concat_zeros)
        return [
            {
                name: np.asarray(out_arrs[i]).reshape(
                    NCORES, *out_shapes[i][0]
                )[c]
                for i, name in enumerate(out_names)
            }
            for c in range(NCORES)
        ]

    _CACHE["runner"] = run
    _CACHE["parts"] = dict(
        nc=nc, sharded=sharded, in_names=in_names, out_names=out_names,
        out_shapes=out_shapes, mesh=mesh, n_params=n_params,
    )
    return run


def prepare_in_maps(x, Wqkv, Wproj, bproj):
    x = np.ascontiguousarray(np.asarray(x, dtype=np.float32))
    Wqkv = np.asarray(Wqkv, dtype=np.float32)
    Wproj = np.asarray(Wproj, dtype=np.float32)
    bp = np.ascontiguousarray(np.asarray(bproj, dtype=np.float32))

    # (b, f*n, d) -> (b*f, p, kt, n) channel-major partition-major tiles
    xt = np.ascontiguousarray(
        x.reshape(B * F, N, 4, 128).transpose(0, 3, 2, 1)
    )
    # [d_in, m] -> [p, kt, m] partition-major tiles
    WqkvT = np.ascontiguousarray(
        Wqkv.T.reshape(4, 128, 3 * D).transpose(1, 0, 2)
    )
    WprojT = np.ascontiguousarray(
        Wproj.T.reshape(4, 128, VD).transpose(1, 0, 2)
    )
    return [
        {
            "xT": np.ascontiguousarray(xt[c * FPC : (c + 1) * FPC]),
            "WqkvT": WqkvT,
            "WprojT": WprojT,
            "bproj": bp,
        }
        for c in range(NCORES)
    ]


def kernel(x, Wqkv, Wproj, bproj, spatial=None, f=None, n=None, **_ignored):
    in_maps = prepare_in_maps(x, Wqkv, Wproj, bproj)
    results = _get_runner()(in_maps)

    y = np.empty((B * F, N, VD), dtype=np.float32)
    for c in range(NCORES):
        y[c * FPC : (c + 1) * FPC] = results[c]["yT"].transpose(0, 2, 1)
    return y.reshape(B, F * N, VD)


# revision 21
# speedup vs baseline: 52.5264x; 1.0135x over previous
"""Trainium2 Bass kernel for factorized spatial attention (nn_Attention_50379966382361).

Reference computation (per batch b, frame f):
    qkv = x @ Wqkv.T ; split into q,k,v heads (8 heads, hd=64)
    attn = softmax(q @ k.T * hd**-0.5) over spatial tokens (n=784) within the frame
    out  = attn @ v ; merge heads ; y = out @ Wproj.T + bproj

Sharding: data-parallel over the 32 (b, f) frames -> 4 frames per core, weights
replicated.  All tensors are staged channel-major on device (tokens on the free
axis) so no on-device transposes are needed:
  - scoresT[j, i] is computed directly via lhsT=kT, rhs=qT (2 heads row-packed
    in the 128x128 PE array since hd=64).
  - exp via ScalarE with the 1/sqrt(hd) scale folded into the activation scale
    (no max subtraction: |scale*s| < ~1.5 for these input statistics).
  - attn@v uses lhsT=v(token-major, produced directly by a second QKV matmul
    orientation), rhs=expT, 2 heads column-packed.
  - softmax denominators via a ones-column matmul into spare PSUM partitions;
    1/sums broadcast across partitions with a tiny K=33 indicator matmul.
"""

import os

import numpy as np

import concourse.bass as bass
import concourse.mybir as mybir
import concourse.tile as tile
from concourse.bass_utils import run_bass_kernel_spmd

B, F, N, VD, D, H = 2, 16, 784, 512, 512, 8
HD = D // H                      # 64
NCORES = 8
FPC = (B * F) // NCORES          # frames per core = 4
JT = 112                         # token tile (7 * 112 = 784, no tail)
NJ = N // JT                     # 7
CHUNKS = ((0, 512), (512, 272))  # free-dim chunks of 784 (PSUM bank = 512 fp32)
FP32 = mybir.dt.float32
F32R = mybir.dt.float32r
AF = mybir.ActivationFunctionType


def _split_ctrl_waits(nc):
    """This walrus build only accepts a single sync-wait per instruction
    (setupSyncWait raises "Too many sync wait commands"), while Tile's
    scheduler aggregates several.  Move the excess waits onto NoOps inserted
    just before (same engine; engines execute in order, so waiting earlier
    on the same queue is equivalent)."""
    for f in nc.m.functions:
        for blk in f.blocks:
            new_list, changed = [], False
            for inst in blk.instructions:
                si = inst.sync_info
                if si is not None and len(si.on_wait) > 1:
                    waits = list(si.on_wait)
                    for w_i, w in enumerate(waits[:-1]):
                        new_list.append(
                            mybir.InstNoOp(
                                name=f"{inst.name}-waitsplit{w_i}",
                                ins=[],
                                outs=[],
                                engine=inst.engine,
                                bass_nofuse=True,
                                sync_info=mybir.SyncInfo(on_wait=[w], on_update=[]),
                            )
                        )
                    inst.sync_info = mybir.SyncInfo(
                        on_wait=[waits[-1]], on_update=list(si.on_update)
                    )
                    changed = True
                new_list.append(inst)
            if changed:
                blk.instructions = new_list


def build_nc():
    nc = bass.Bass("TRN2", target_bir_lowering=False, debug=False, num_devices=NCORES)

    # host pre-arranges inputs into the on-chip tile layout (partition-major)
    # so every load is a dense contiguous DMA (SWDGE descriptor gen is the
    # startup bottleneck otherwise)
    xT = nc.declare_dram_parameter("xT", [FPC, 4, 128, N], FP32, isOutput=False)
    WqkvT = nc.declare_dram_parameter("WqkvT", [128, 4, 3 * D], FP32, isOutput=False)
    WprojT = nc.declare_dram_parameter("WprojT", [128, 4, VD], FP32, isOutput=False)
    bproj = nc.declare_dram_parameter("bproj", [VD], FP32, isOutput=False)
    yT = nc.declare_dram_parameter("yT", [FPC, VD, N], FP32, isOutput=True)

    # attn@v output columns: head A of a pair at [0, 784), head B at
    # [1024, 1808) of a [65, 2048] psum tile (PSUM-bank aligned chunks).
    BOFF = 1024

    with tile.TileContext(nc) as tc:
        with (
            nc.allow_low_precision(
                reason="float32r matmul operands (TF32-like, ~1.7e-4 rel err)"
            ),
            tc.tile_pool(name="w", bufs=1) as w_pool,
            tc.tile_pool(name="x", bufs=2) as x_pool,
            tc.tile_pool(name="qk", bufs=4) as qk_pool,
            tc.tile_pool(name="v", bufs=10) as v_pool,
            tc.tile_pool(name="e", bufs=9) as e_pool,
            tc.tile_pool(name="osc", bufs=2) as osc_pool,
            tc.tile_pool(name="pb", bufs=5) as pb_pool,
            tc.tile_pool(name="r", bufs=2) as r_pool,
            tc.tile_pool(name="y", bufs=3) as y_pool,
            tc.tile_pool(name="mm", bufs=2, space="PSUM") as mm_ps,
            tc.tile_pool(name="oacc", bufs=1, space="PSUM") as out_ps,
        ):
            # ---- constants / weights (once per core) ----
            # (emitted as four per-k-tile DMAs so frame-0 matmuls can start
            # as soon as their k-slice has landed)
            W1 = w_pool.tile([128, 4, 3 * D], F32R)   # WqkvT, d-major tiles
            for kt in range(4):
                nc.gpsimd.dma_start(out=W1[:, kt, :], in_=WqkvT[:, kt, :])
            W2 = w_pool.tile([128, 4, VD], F32R)      # WprojT, d-major tiles
            nc.gpsimd.dma_start(out=W2, in_=WprojT[:])
            bias_sb = w_pool.tile([128, 4], FP32)
            nc.sync.dma_start(out=bias_sb, in_=bproj.rearrange("(a p) -> p a", p=128))
            # fp32 staging for constants (memset cannot produce float32r)
            ones_f = w_pool.tile([JT, 8, 1], FP32)
            nc.vector.memset(ones_f, 1.0)
            # K=1 lhsT for the 1/sums partition-broadcast: row 64 of [65, 64]
            o65_f = w_pool.tile([65, HD], FP32)
            nc.vector.memset(o65_f, 0.0)
            nc.vector.memset(o65_f[64:65, :], 1.0)
            ones65 = w_pool.tile([65, HD], F32R)
            nc.vector.tensor_copy(ones65, o65_f)

            # KERNEL_TIME_REPS>1 repeats the whole computation (identical
            # output) so wall-clock deltas can isolate device time
            reps = int(os.environ.get("KERNEL_TIME_REPS", "1"))
            pending_proj = None
            for fr in [f for _ in range(reps) for f in range(FPC)]:
                # ---- load xT for this frame, channel-major ----
                X = x_pool.tile([128, 4, N], F32R, tag="X")
                for kt in range(4):
                    nc.gpsimd.dma_start(out=X[:, kt, :], in_=xT[fr, kt])

                # ---- V token-major with a ones column per head:
                #      vt[:, 65h : 65h+64] = v_h, vt[:, 65h+64] = 1.0 ----
                v_tok = []
                for tt in range(NJ):
                    psv = mm_ps.tile([JT, VD], FP32, tag="mm")
                    for kt in range(4):
                        nc.tensor.matmul(
                            psv,
                            X[:, kt, tt * JT : (tt + 1) * JT],
                            W1[:, kt, 2 * D : 3 * D],
                            start=(kt == 0),
                            stop=(kt == 3),
                        )
                    vt = v_pool.tile([JT, H, 65], F32R, tag="vtok")
                    nc.vector.tensor_copy(
                        vt[:, :, 0:HD], psv.rearrange("p (h c) -> p h c", c=HD)
                    )
                    nc.vector.tensor_copy(vt[:, :, HD : HD + 1], ones_f)
                    v_tok.append(vt)

                def emit_proj(pfr, osc_list):
                    # output projection + bias for frame pfr (deferred so the
                    # next frame's V matmuls fill the PE queue while the last
                    # pair's normalization drains on DVE)
                    for ot in range(4):
                        ps_y = mm_ps.tile([128, N], FP32, tag="mm")
                        for kt in range(4):
                            for c0, cw in CHUNKS:
                                nc.tensor.matmul(
                                    ps_y[:, c0 : c0 + cw],
                                    W2[:, kt, ot * 128 : (ot + 1) * 128],
                                    osc_list[kt][:, c0 : c0 + cw],
                                    start=(kt == 0),
                                    stop=(kt == 3),
                                )
                        yt = y_pool.tile([128, N], FP32, tag="yT")
                        # bias-add on ScalarE: ACT idles at frame boundaries
                        # while DVE is the local bottleneck
                        nc.scalar.activation(
                            out=yt,
                            in_=ps_y,
                            func=AF.Identity,
                            bias=bias_sb[:, ot : ot + 1],
                        )
                        nc.sync.dma_start(
                            out=yT[pfr, ot * 128 : (ot + 1) * 128, :], in_=yt
                        )

                if pending_proj is not None:
                    emit_proj(*pending_proj)
                    pending_proj = None

                def emit_attnv(p, jt, e_ab, out_acc):
                    for h in range(2):
                        for c0, cw in CHUNKS:
                            nc.tensor.matmul(
                                out_acc[0:65, BOFF * h + c0 : BOFF * h + c0 + cw],
                                v_tok[jt][:, 2 * p + h, :],
                                e_ab[h][:, c0 : c0 + cw],
                                start=(jt == 0),
                                stop=(jt == NJ - 1),
                            )

                def emit_normalize(p, out_acc):
                    # rows 0:64 * (1/row 64), both heads of the pair
                    r_sb = r_pool.tile([65, 2, N], F32R, tag="rsb")
                    nc.vector.reciprocal(
                        out=r_sb[64:65, :, :],
                        in_=out_acc[64:65, :].rearrange("p (s q) -> p s q", s=2)[
                            :, :, 0:N
                        ],
                    )
                    r2 = r_pool.tile([HD, 2, N], FP32, tag="r2")
                    for h in range(2):
                        ps_r = mm_ps.tile([HD, N], FP32, tag="mm")
                        for c0, cw in CHUNKS:
                            nc.tensor.matmul(
                                ps_r[:, c0 : c0 + cw],
                                ones65[64:65, :],
                                r_sb[64:65, h, c0 : c0 + cw],
                                start=True,
                                stop=True,
                            )
                        nc.vector.tensor_copy(r2[:, h, :], ps_r)
                    big = pb_pool.tile([128, N], F32R, tag="pb")
                    nc.vector.tensor_mul(
                        big[0:HD, :], out_acc[0:HD, 0:N], r2[:, 0, :]
                    )
                    odd = osc_pool.tile([HD, N], F32R, tag="osc")
                    nc.vector.tensor_mul(
                        odd, out_acc[0:HD, BOFF : BOFF + N], r2[:, 1, :]
                    )
                    nc.sync.dma_start(out=big[HD:128, :], in_=odd)
                    out_osc.append(big)

                out_osc = []
                prev_pair = None  # (p, out_acc) awaiting normalize
                for p in range(4):  # head pairs (2p, 2p+1)
                    # ---- q/k channel-major tiles for this pair ----
                    qk = {}
                    for name, ot in (("q", p), ("k", 4 + p)):
                        ps = mm_ps.tile([128, N], FP32, tag="mm")
                        for kt in range(4):
                            for c0, cw in CHUNKS:
                                nc.tensor.matmul(
                                    ps[:, c0 : c0 + cw],
                                    W1[:, kt, ot * 128 : (ot + 1) * 128],
                                    X[:, kt, c0 : c0 + cw],
                                    start=(kt == 0),
                                    stop=(kt == 3),
                                )
                        t = qk_pool.tile([128, N], F32R, tag="qkT")
                        nc.vector.tensor_copy(t, ps)
                        qk[name] = t

                    if prev_pair is not None:
                        emit_normalize(*prev_pair)
                        prev_pair = None

                    # ---- attention: out rows 0:64 = attn@v, row 64 = sums ----
                    # software-pipelined: attn@v for j-tile jt-1 is emitted
                    # after scores/exp of jt so the in-order PE queue never
                    # heads-of-line blocks on the exp latency.
                    out_acc = out_ps.tile([65, 2048], FP32, tag="oacc")
                    pend = None
                    for jt in range(NJ):
                        j0 = jt * JT
                        e_ab = []
                        for h in range(2):  # head within pair
                            hp = h * HD
                            ps_s = mm_ps.tile([JT, N], FP32, tag="mm")
                            for c0, cw in CHUNKS:
                                nc.tensor.matmul(
                                    ps_s[:, c0 : c0 + cw],
                                    qk["k"][hp : hp + HD, j0 : j0 + JT],
                                    qk["q"][hp : hp + HD, c0 : c0 + cw],
                                    start=True,
                                    stop=True,
                                    tile_position=(hp, 0),
                                )
                            e = e_pool.tile([JT, N], F32R, tag="expT")
                            nc.scalar.activation(
                                out=e, in_=ps_s, func=AF.Exp, scale=HD**-0.5
                            )
                            e_ab.append(e)
                        if pend is not None:
                            emit_attnv(p, pend[0], pend[1], out_acc)
                        pend = (jt, e_ab)
                    emit_attnv(p, pend[0], pend[1], out_acc)
                    prev_pair = (p, out_acc)
                emit_normalize(*prev_pair)

                pending_proj = (fr, out_osc)
            emit_proj(*pending_proj)

    _split_ctrl_waits(nc)
    return nc


_CACHE = {}


def _get_runner():
    """Build the Bass module once and wrap it in a cached sharded jax.jit
    callable (replicates bass2jax.run_bass_via_pjrt but reusable across
    calls, so repeated invocations don't re-lower/re-compile)."""
    if "runner" in _CACHE:
        return _CACHE["runner"]

    import jax
    from jax.experimental.shard_map import shard_map
    from jax.sharding import Mesh, PartitionSpec
    from concourse import bass2jax, mybir as _mybir

    nc = build_nc()
    bass2jax.install_neuronx_cc_hook()
    assert nc.dbg_addr is None
    partition_name = nc.partition_id_tensor.name if nc.partition_id_tensor else None

    in_names, out_names, out_avals, out_shapes = [], [], [], []
    for alloc in nc.m.functions[0].allocations:
        if not isinstance(alloc, _mybir.MemoryLocationSet):
            continue
        name = alloc.memorylocations[0].name
        if alloc.kind == "ExternalInput":
            if name != partition_name:
                in_names.append(name)
        elif alloc.kind == "ExternalOutput":
            shape = tuple(alloc.tensor_shape)
            dtype = _mybir.dt.np(alloc.dtype)
            out_names.append(name)
            out_avals.append(jax.core.ShapedArray(shape, dtype))
            out_shapes.append((shape, dtype))
    n_params = len(in_names)
    all_names = in_names + out_names
    if partition_name is not None:
        all_names = all_names + [partition_name]

    def _body(*args):
        operands = list(args)
        if partition_name is not None:
            operands.append(bass2jax.partition_id_tensor())
        outs = bass2jax._bass_exec_p.bind(
            *operands,
            out_avals=tuple(out_avals),
            in_names=tuple(all_names),
            out_names=tuple(out_names),
            lowering_input_output_aliases=(),
            sim_require_finite=True,
            sim_require_nnan=True,
            nc=nc,
        )
        return tuple(outs)

    devices = jax.devices()[:NCORES]
    mesh = Mesh(np.asarray(devices), ("core",))
    nin = n_params + len(out_names)
    sharded = jax.jit(
        shard_map(
            _body,
            mesh=mesh,
            in_specs=(PartitionSpec("core"),) * nin,
            out_specs=(PartitionSpec("core"),) * len(out_names),
            check_rep=False,
        ),
        donate_argnums=tuple(range(n_params, nin)),
        keep_unused=True,
    )

    def run(in_maps):
        concat_in = [
            np.concatenate([np.asarray(m[name]) for m in in_maps], axis=0)
            for name in in_names
        ]
        concat_zeros = [
            np.zeros((NCORES * s[0], *s[1:]), dt) for s, dt in out_shapes
        ]
        out_arrs = sharded(*concat_in, *concat_zeros)
        return [
            {
                name: np.asarray(out_arrs[i]).reshape(
                    NCORES, *out_shapes[i][0]
                )[c]
                for i, name in enumerate(out_names)
            }
            for c in range(NCORES)
        ]

    _CACHE["runner"] = run
    _CACHE["parts"] = dict(
        nc=nc, sharded=sharded, in_names=in_names, out_names=out_names,
        out_shapes=out_shapes, mesh=mesh, n_params=n_params,
    )
    return run


def prepare_in_maps(x, Wqkv, Wproj, bproj):
    x = np.ascontiguousarray(np.asarray(x, dtype=np.float32))
    Wqkv = np.asarray(Wqkv, dtype=np.float32)
    Wproj = np.asarray(Wproj, dtype=np.float32)
    bp = np.ascontiguousarray(np.asarray(bproj, dtype=np.float32))

    # (b, f*n, d) -> (b*f, kt, p, n) channel-major tiles, kt-major so each
    # k-tile is one dense contiguous DMA
    xt = np.ascontiguousarray(
        x.reshape(B * F, N, 4, 128).transpose(0, 2, 3, 1)
    )
    # [d_in, m] -> [p, kt, m] partition-major tiles
    WqkvT = np.ascontiguousarray(
        Wqkv.T.reshape(4, 128, 3 * D).transpose(1, 0, 2)
    )
    WprojT = np.ascontiguousarray(
        Wproj.T.reshape(4, 128, VD).transpose(1, 0, 2)
    )
    return [
        {
            "xT": np.ascontiguousarray(xt[c * FPC : (c + 1) * FPC]),
            "WqkvT": WqkvT,
            "WprojT": WprojT,
            "bproj": bp,
        }
        for c in range(NCORES)
    ]


def kernel(x, Wqkv, Wproj, bproj, spatial=None, f=None, n=None, **_ignored):
    in_maps = prepare_in_maps(x, Wqkv, Wproj, bproj)
    results = _get_runner()(in_maps)

    y = np.empty((B * F, N, VD), dtype=np.float32)
    for c in range(NCORES):
        y[c * FPC : (c + 1) * FPC] = results[c]["yT"].transpose(0, 2, 1)
    return y.reshape(B, F * N, VD)


# revision 26
# speedup vs baseline: 10692.4250x; 203.5629x over previous
"""Trainium2 Bass kernel for factorized spatial attention (nn_Attention_50379966382361).

Reference computation (per batch b, frame f):
    qkv = x @ Wqkv.T ; split into q,k,v heads (8 heads, hd=64)
    attn = softmax(q @ k.T * hd**-0.5) over spatial tokens (n=784) within the frame
    out  = attn @ v ; merge heads ; y = out @ Wproj.T + bproj

Sharding: data-parallel over the 32 (b, f) frames -> 4 frames per core, weights
replicated.  All tensors are staged channel-major on device (tokens on the free
axis) so no on-device transposes are needed:
  - scoresT[j, i] is computed directly via lhsT=kT, rhs=qT (2 heads row-packed
    in the 128x128 PE array since hd=64).
  - exp via ScalarE with the 1/sqrt(hd) scale folded into the activation scale
    (no max subtraction: |scale*s| < ~1.5 for these input statistics).
  - attn@v uses lhsT=v (token-major, produced directly by a second QKV
    matmul orientation) with a ones column appended per head, so output row 64
    accumulates the softmax denominators for free (M=65).
  - 1/sums is broadcast across partitions with a tiny K=1 matmul; everything
    writes PSUM partition 0 (this walrus rejects matmul dst partitions != 0).
"""

import os

import numpy as np

import concourse.bass as bass
import concourse.mybir as mybir
import concourse.tile as tile

B, F, N, VD, D, H = 2, 16, 784, 512, 512, 8
HD = D // H                      # 64
NCORES = 8
FPC = (B * F) // NCORES          # frames per core = 4
JT = 112                         # token tile (7 * 112 = 784, no tail)
PIPE_DEPTH = 2                   # attn@v trails scores/exp by this many j-tiles
NJ = N // JT                     # 7
CHUNKS = ((0, 512), (512, 272))  # free-dim chunks of 784 (PSUM bank = 512 fp32)
FP32 = mybir.dt.float32
F32R = mybir.dt.float32r
AF = mybir.ActivationFunctionType


def _split_ctrl_waits(nc):
    """This walrus build only accepts a single sync-wait per instruction
    (setupSyncWait raises "Too many sync wait commands"), while Tile's
    scheduler aggregates several.  Move the excess waits onto NoOps inserted
    just before (same engine; engines execute in order, so waiting earlier
    on the same queue is equivalent)."""
    for f in nc.m.functions:
        for blk in f.blocks:
            new_list, changed = [], False
            for inst in blk.instructions:
                si = inst.sync_info
                if si is not None and len(si.on_wait) > 1:
                    waits = list(si.on_wait)
                    for w_i, w in enumerate(waits[:-1]):
                        new_list.append(
                            mybir.InstNoOp(
                                name=f"{inst.name}-waitsplit{w_i}",
                                ins=[],
                                outs=[],
                                engine=inst.engine,
                                bass_nofuse=True,
                                sync_info=mybir.SyncInfo(on_wait=[w], on_update=[]),
                            )
                        )
                    inst.sync_info = mybir.SyncInfo(
                        on_wait=[waits[-1]], on_update=list(si.on_update)
                    )
                    changed = True
                new_list.append(inst)
            if changed:
                blk.instructions = new_list


def build_nc():
    nc = bass.Bass("TRN2", target_bir_lowering=False, debug=False, num_devices=NCORES)

    # host pre-arranges inputs into the on-chip tile layout (partition-major)
    # so every load is a dense contiguous DMA (SWDGE descriptor gen is the
    # startup bottleneck otherwise)
    xT = nc.declare_dram_parameter("xT", [FPC, 4, 128, N], FP32, isOutput=False)
    WqkvT = nc.declare_dram_parameter("WqkvT", [128, 4, 3 * D], FP32, isOutput=False)
    WprojT = nc.declare_dram_parameter("WprojT", [128, 4, VD], FP32, isOutput=False)
    bproj = nc.declare_dram_parameter("bproj", [VD], FP32, isOutput=False)
    yT = nc.declare_dram_parameter("yT", [FPC, VD, N], FP32, isOutput=True)

    # attn@v output columns: head A of a pair at [0, 784), head B at
    # [1024, 1808) of a [65, 2048] psum tile (PSUM-bank aligned chunks).
    BOFF = 1024

    with tile.TileContext(nc) as tc:
        with (
            nc.allow_low_precision(
                reason="float32r matmul operands (TF32-like, ~1.7e-4 rel err)"
            ),
            tc.tile_pool(name="w", bufs=1) as w_pool,
            tc.tile_pool(name="x", bufs=2) as x_pool,
            tc.tile_pool(name="qk", bufs=4) as qk_pool,
            tc.tile_pool(name="v", bufs=10) as v_pool,
            tc.tile_pool(name="e", bufs=9) as e_pool,
            tc.tile_pool(name="osc", bufs=2) as osc_pool,
            tc.tile_pool(name="pb", bufs=5) as pb_pool,
            tc.tile_pool(name="r", bufs=2) as r_pool,
            tc.tile_pool(name="y", bufs=3) as y_pool,
            tc.tile_pool(name="mm", bufs=2, space="PSUM") as mm_ps,
            tc.tile_pool(name="oacc", bufs=1, space="PSUM") as out_ps,
        ):
            # ---- constants / weights (once per core) ----
            # (emitted as four per-k-tile DMAs so frame-0 matmuls can start
            # as soon as their k-slice has landed)
            W1 = w_pool.tile([128, 4, 3 * D], F32R)   # WqkvT, d-major tiles
            for kt in range(4):
                nc.gpsimd.dma_start(out=W1[:, kt, :], in_=WqkvT[:, kt, :])
            W2 = w_pool.tile([128, 4, VD], F32R)      # WprojT, d-major tiles
            nc.gpsimd.dma_start(out=W2, in_=WprojT[:])
            bias_sb = w_pool.tile([128, 4], FP32)
            nc.sync.dma_start(out=bias_sb, in_=bproj.rearrange("(a p) -> p a", p=128))
            # fp32 staging for constants (memset cannot produce float32r)
            ones_f = w_pool.tile([JT, 8, 1], FP32)
            nc.vector.memset(ones_f, 1.0)
            # K=1 lhsT for the 1/sums partition-broadcast: row 64 of [65, 64]
            o65_f = w_pool.tile([65, HD], FP32)
            nc.vector.memset(o65_f, 0.0)
            nc.vector.memset(o65_f[64:65, :], 1.0)
            ones65 = w_pool.tile([65, HD], F32R)
            nc.vector.tensor_copy(ones65, o65_f)

            # KERNEL_TIME_REPS>1 repeats the whole computation (identical
            # output) so wall-clock deltas can isolate device time
            reps = int(os.environ.get("KERNEL_TIME_REPS", "1"))
            pending_proj = None
            for fr in [f for _ in range(reps) for f in range(FPC)]:
                # ---- load xT for this frame, channel-major ----
                X = x_pool.tile([128, 4, N], F32R, tag="X")
                for kt in range(4):
                    nc.gpsimd.dma_start(out=X[:, kt, :], in_=xT[fr, kt])

                # ---- V token-major with a ones column per head:
                #      vt[:, 65h : 65h+64] = v_h, vt[:, 65h+64] = 1.0 ----
                v_tok = []
                for tt in range(NJ):
                    psv = mm_ps.tile([JT, VD], FP32, tag="mm")
                    for kt in range(4):
                        nc.tensor.matmul(
                            psv,
                            X[:, kt, tt * JT : (tt + 1) * JT],
                            W1[:, kt, 2 * D : 3 * D],
                            start=(kt == 0),
                            stop=(kt == 3),
                        )
                    vt = v_pool.tile([JT, H, 65], F32R, tag="vtok")
                    nc.vector.tensor_copy(
                        vt[:, :, 0:HD], psv.rearrange("p (h c) -> p h c", c=HD)
                    )
                    nc.vector.tensor_copy(vt[:, :, HD : HD + 1], ones_f)
                    v_tok.append(vt)

                def emit_proj(pfr, osc_list):
                    # output projection + bias for frame pfr (deferred so the
                    # next frame's V matmuls fill the PE queue while the last
                    # pair's normalization drains on DVE)
                    for ot in range(4):
                        ps_y = mm_ps.tile([128, N], FP32, tag="mm")
                        for kt in range(4):
                            for c0, cw in CHUNKS:
                                nc.tensor.matmul(
                                    ps_y[:, c0 : c0 + cw],
                                    W2[:, kt, ot * 128 : (ot + 1) * 128],
                                    osc_list[kt][:, c0 : c0 + cw],
                                    start=(kt == 0),
                                    stop=(kt == 3),
                                )
                        yt = y_pool.tile([128, N], FP32, tag="yT")
                        # bias-add on ScalarE: ACT idles at frame boundaries
                        # while DVE is the local bottleneck
                        nc.scalar.activation(
                            out=yt,
                            in_=ps_y,
                            func=AF.Identity,
                            bias=bias_sb[:, ot : ot + 1],
                        )
                        nc.sync.dma_start(
                            out=yT[pfr, ot * 128 : (ot + 1) * 128, :], in_=yt
                        )

                if pending_proj is not None:
                    emit_proj(*pending_proj)
                    pending_proj = None

                def emit_attnv(p, jt, e_ab, out_acc):
                    for h in range(2):
                        for c0, cw in CHUNKS:
                            nc.tensor.matmul(
                                out_acc[0:65, BOFF * h + c0 : BOFF * h + c0 + cw],
                                v_tok[jt][:, 2 * p + h, :],
                                e_ab[h][:, c0 : c0 + cw],
                                start=(jt == 0),
                                stop=(jt == NJ - 1),
                            )

                def emit_recip(p, out_acc):
                    # 1/sums as soon as the last attn@v matmul lands, ahead of
                    # any other queued DVE work
                    r_sb = r_pool.tile([65, 2, N], F32R, tag="rsb")
                    nc.vector.reciprocal(
                        out=r_sb[64:65, :, :],
                        in_=out_acc[64:65, :].rearrange("p (s q) -> p s q", s=2)[
                            :, :, 0:N
                        ],
                    )
                    return r_sb

                def emit_normalize(p, out_acc, r_sb):
                    # rows 0:64 * (1/row 64), both heads of the pair
                    r2 = r_pool.tile([HD, 2, N], FP32, tag="r2")
                    for h in range(2):
                        ps_r = mm_ps.tile([HD, N], FP32, tag="mm")
                        for c0, cw in CHUNKS:
                            nc.tensor.matmul(
                                ps_r[:, c0 : c0 + cw],
                                ones65[64:65, :],
                                r_sb[64:65, h, c0 : c0 + cw],
                                start=True,
                                stop=True,
                            )
                        nc.vector.tensor_copy(r2[:, h, :], ps_r)
                    big = pb_pool.tile([128, N], F32R, tag="pb")
                    nc.vector.tensor_mul(
                        big[0:HD, :], out_acc[0:HD, 0:N], r2[:, 0, :]
                    )
                    odd = osc_pool.tile([HD, N], F32R, tag="osc")
                    nc.vector.tensor_mul(
                        odd, out_acc[0:HD, BOFF : BOFF + N], r2[:, 1, :]
                    )
                    nc.sync.dma_start(out=big[HD:128, :], in_=odd)
                    out_osc.append(big)

                out_osc = []
                prev_pair = None  # (p, out_acc, r_sb) awaiting normalize
                qk_tiles = {}

                def emit_qk(p):
                    # q/k channel-major tiles for pair p
                    qk = {}
                    for name, ot in (("q", p), ("k", 4 + p)):
                        ps = mm_ps.tile([128, N], FP32, tag="mm")
                        for kt in range(4):
                            for c0, cw in CHUNKS:
                                nc.tensor.matmul(
                                    ps[:, c0 : c0 + cw],
                                    W1[:, kt, ot * 128 : (ot + 1) * 128],
                                    X[:, kt, c0 : c0 + cw],
                                    start=(kt == 0),
                                    stop=(kt == 3),
                                )
                        t = qk_pool.tile([128, N], F32R, tag="qkT")
                        nc.vector.tensor_copy(t, ps)
                        qk[name] = t
                    qk_tiles[p] = qk

                for p in range(4):  # head pairs (2p, 2p+1)
                    emit_qk(p)
                    qk = qk_tiles.pop(p)

                    if prev_pair is not None:
                        emit_normalize(*prev_pair)
                        prev_pair = None

                    # ---- attention: out rows 0:64 = attn@v, row 64 = sums ----
                    # software-pipelined: attn@v for j-tile jt-1 is emitted
                    # after scores/exp of jt so the in-order PE queue never
                    # heads-of-line blocks on the exp latency.
                    out_acc = out_ps.tile([65, 2048], FP32, tag="oacc")
                    pend = []
                    for jt in range(NJ):
                        j0 = jt * JT
                        e_ab = []
                        for h in range(2):  # head within pair
                            hp = h * HD
                            ps_s = mm_ps.tile([JT, N], FP32, tag="mm")
                            for c0, cw in CHUNKS:
                                nc.tensor.matmul(
                                    ps_s[:, c0 : c0 + cw],
                                    qk["k"][hp : hp + HD, j0 : j0 + JT],
                                    qk["q"][hp : hp + HD, c0 : c0 + cw],
                                    start=True,
                                    stop=True,
                                    tile_position=(hp, 0),
                                )
                            e = e_pool.tile([JT, N], F32R, tag="expT")
                            nc.scalar.activation(
                                out=e, in_=ps_s, func=AF.Exp, scale=HD**-0.5
                            )
                            e_ab.append(e)
                        pend.append((jt, e_ab))
                        if len(pend) > PIPE_DEPTH:
                            jt_, e_ = pend.pop(0)
                            emit_attnv(p, jt_, e_, out_acc)
                    for jt_, e_ in pend:
                        emit_attnv(p, jt_, e_, out_acc)
                    r_sb = emit_recip(p, out_acc)
                    prev_pair = (p, out_acc, r_sb)
                emit_normalize(*prev_pair)

                pending_proj = (fr, out_osc)
            emit_proj(*pending_proj)

    _split_ctrl_waits(nc)
    return nc


_CACHE = {}


def _get_runner():
    """Build the Bass module once and wrap it in a cached sharded jax.jit
    callable (replicates bass2jax.run_bass_via_pjrt but reusable across
    calls, so repeated invocations don't re-lower/re-compile)."""
    if "runner" in _CACHE:
        return _CACHE["runner"]

    import jax
    from jax.experimental.shard_map import shard_map
    from jax.sharding import Mesh, PartitionSpec
    from concourse import bass2jax, mybir as _mybir

    nc = build_nc()
    bass2jax.install_neuronx_cc_hook()
    assert nc.dbg_addr is None
    partition_name = nc.partition_id_tensor.name if nc.partition_id_tensor else None

    in_names, out_names, out_avals, out_shapes = [], [], [], []
    for alloc in nc.m.functions[0].allocations:
        if not isinstance(alloc, _mybir.MemoryLocationSet):
            continue
        name = alloc.memorylocations[0].name
        if alloc.kind == "ExternalInput":
            if name != partition_name:
                in_names.append(name)
        elif alloc.kind == "ExternalOutput":
            shape = tuple(alloc.tensor_shape)
            dtype = _mybir.dt.np(alloc.dtype)
            out_names.append(name)
            out_avals.append(jax.core.ShapedArray(shape, dtype))
            out_shapes.append((shape, dtype))
    n_params = len(in_names)
    all_names = in_names + out_names
    if partition_name is not None:
        all_names = all_names + [partition_name]

    def _body(*args):
        operands = list(args)
        if partition_name is not None:
            operands.append(bass2jax.partition_id_tensor())
        outs = bass2jax._bass_exec_p.bind(
            *operands,
            out_avals=tuple(out_avals),
            in_names=tuple(all_names),
            out_names=tuple(out_names),
            lowering_input_output_aliases=(),
            sim_require_finite=True,
            sim_require_nnan=True,
            nc=nc,
        )
        return tuple(outs)

    devices = jax.devices()[:NCORES]
    mesh = Mesh(np.asarray(devices), ("core",))
    nin = n_params + len(out_names)
    sharded = jax.jit(
        shard_map(
            _body,
            mesh=mesh,
            in_specs=(PartitionSpec("core"),) * nin,
            out_specs=(PartitionSpec("core"),) * len(out_names),
            check_rep=False,
        ),
        donate_argnums=tuple(range(n_params, nin)),
        keep_unused=True,
    )

    def run(in_maps):
        concat_in = [
            np.concatenate([np.asarray(m[name]) for m in in_maps], axis=0)
            for name in in_names
        ]
        concat_zeros = [
            np.zeros((NCORES * s[0], *s[1:]), dt) for s, dt in out_shapes
        ]
        out_arrs = sharded(*concat_in, *concat_zeros)
        return [
            {
                name: np.asarray(out_arrs[i]).reshape(
                    NCORES, *out_shapes[i][0]
                )[c]
                for i, name in enumerate(out_names)
            }
            for c in range(NCORES)
        ]

    _CACHE["runner"] = run
    _CACHE["parts"] = dict(
        nc=nc, sharded=sharded, in_names=in_names, out_names=out_names,
        out_shapes=out_shapes, mesh=mesh, n_params=n_params,
    )
    return run


def prepare_in_maps(x, Wqkv, Wproj, bproj):
    x = np.ascontiguousarray(np.asarray(x, dtype=np.float32))
    Wqkv = np.asarray(Wqkv, dtype=np.float32)
    Wproj = np.asarray(Wproj, dtype=np.float32)
    bp = np.ascontiguousarray(np.asarray(bproj, dtype=np.float32))

    # (b, f*n, d) -> (b*f, kt, p, n) channel-major tiles, kt-major so each
    # k-tile is one dense contiguous DMA
    xt = np.ascontiguousarray(
        x.reshape(B * F, N, 4, 128).transpose(0, 2, 3, 1)
    )
    # [d_in, m] -> [p, kt, m] partition-major tiles
    WqkvT = np.ascontiguousarray(
        Wqkv.T.reshape(4, 128, 3 * D).transpose(1, 0, 2)
    )
    WprojT = np.ascontiguousarray(
        Wproj.T.reshape(4, 128, VD).transpose(1, 0, 2)
    )
    return [
        {
            "xT": np.ascontiguousarray(xt[c * FPC : (c + 1) * FPC]),
            "WqkvT": WqkvT,
            "WprojT": WprojT,
            "bproj": bp,
        }
        for c in range(NCORES)
    ]


def kernel(x, Wqkv, Wproj, bproj, spatial=None, f=None, n=None, **_ignored):
    in_maps = prepare_in_maps(x, Wqkv, Wproj, bproj)
    results = _get_runner()(in_maps)

    y = np.empty((B * F, N, VD), dtype=np.float32)
    for c in range(NCORES):
        y[c * FPC : (c + 1) * FPC] = results[c]["yT"].transpose(0, 2, 1)
    return y.reshape(B, F * N, VD)

